# revision 13
# baseline (speedup 1.0000x reference)
"""AlphaNet_v1 Trainium2 kernel — single fused NEFF, 8-core data parallel.

Host side: x~ = (x-1) as fp16 (centering shifts are BN-invariant), w-major
(row, w, f, d); BN stats from a 16384-row sample via numpy; BN folded into
MLP weights; W1 permuted to the device F-column layout.

Device (per core, per 512-row iteration, all fp16):
  P [128,(g,w,54,10)] products: diag squares (Act), 36 pair products (DVE,
  8 offset-batched TT), day-weight products (Pool). Halving-tree d-reduce
  (DVE, final stage Pool) -> S [g,w,54]. msum tree on DVE. Derived
  features with fused tensor_scalar / scalar_tensor_tensor ops into a
  4B-aligned (w,124)-blocked F [128, 640] per chunk; window max/min on
  DVE. F transposed by the DMA engines (XBAR transpose, one call/iter)
  into Ft [128,(g,5,128)]; fused MLP on PE (5 accumulating matmuls +
  relu + W2) -> out [16384] f32.

Stored-feature units (ref = s*stored + c): corr (1,0), cov (1/9,0),
sd=V*rsqrt(V+kap) (1/3,0), zs=mean*rsqrt(V+kap) (3,0), ret=xl/xf (1,-1),
decay (1/55,+1), m=mean (1,0).
"""

import contextlib
import numpy as np

import bass_rust
import concourse.bass as bass
import concourse.mybir as mybir
import concourse.tile as tile
from concourse.bass_utils import run_bass_kernel_spmd

F32 = mybir.dt.float32
FP16 = mybir.dt.float16
ALU = mybir.AluOpType
AF = mybir.ActivationFunctionType
AX = mybir.AxisListType

NCORES = 8
B_TOTAL = 131072
ROWS = B_TOTAL // NCORES          # 16384
CHUNK = 128
G = 4                             # chunks per iteration
NITER = ROWS // (CHUNK * G)       # 32
NF, NW, ND = 9, 3, 10
NPAIR = 36
NSLOT = 54                        # per (row, window): diag 0:9 | pairs 9:45 | dw 45:54
FPAD = 640
EPS_BN, EPS = 1e-5, 1e-8
KAP = 1e-6
SQ10 = float(np.sqrt(10.0) / 10.0)

I_IDX, J_IDX = np.triu_indices(NF, k=1)
GROUPS = [("corr", 36), ("cov", 36), ("sd", 9), ("zs", 9), ("ret", 9), ("decay", 9), ("m", 9)]
S_C = {"corr": (1.0, 0.0), "cov": (1.0 / 9.0, 0.0), "sd": (1.0 / 3.0, 0.0),
       "zs": (3.0, 0.0), "ret": (1.0, -1.0), "decay": (1.0 / 55.0, 1.0), "m": (1.0, 0.0)}
# F column layout per chunk (all starts 4B-aligned):
#  0:372    3x (w,124) [corr36 | cov36 | sd10 | zs10 | ret10 | dec10 | m10 | pad2]
#  372:496 max124 | 496:620 min124 | 620:640 pad
WBLK = 124
BIG0, MAX0, MIN0 = 0, 372, 496
SFAM = {"sd": 72, "zs": 82, "ret": 92, "decay": 102, "m": 112}

_wsplit_n = [0]


def split_multi_waits(nc):
    for fn in nc.m.functions:
        for bb in fn.blocks:
            new_list = []
            for ins in bb.instructions:
                si = ins.sync_info
                waits = list(si.on_wait) if (si is not None and si.on_wait) else []
                if len(waits) > 1:
                    for w in waits[:-1]:
                        _wsplit_n[0] += 1
                        d = bass_rust.InstDrain(
                            name=f"wsplit-{_wsplit_n[0]}", ins=[], outs=[]
                        )
                        d.engine = ins.engine
                        d.sync_info = mybir.SyncInfo(on_wait=[w], on_update=[])
                        new_list.append(d)
                    si.on_wait = [waits[-1]]
                new_list.append(ins)
            bb.instructions[:] = new_list


def act_raw(nc, out, in_, func, bias_ap, scale):
    sc = nc.scalar
    ins = [
        sc.lower_ap(in_),
        sc.lower_ap(bias_ap),
        mybir.ImmediateValue(dtype=mybir.dt.float32, value=float(scale)),
        mybir.ImmediateValue(dtype=mybir.dt.float32, value=0.0),
    ]
    return sc.add_instruction(
        mybir.InstActivation(
            name=nc.get_next_instruction_name(),
            func=func,
            ins=ins,
            outs=[sc.lower_ap(out)],
        )
    )


# ---------------------------------------------------------------------------
def build_neff():
    nc = bass.Bass()
    x_ext = nc.declare_dram_parameter("x16", [ROWS, 270], FP16, isOutput=False)
    w1_ext = nc.declare_dram_parameter("w1t", [FPAD, 30], FP16, isOutput=False)
    b1_ext = nc.declare_dram_parameter("b1p", [30, 1], F32, isOutput=False)
    w2_ext = nc.declare_dram_parameter("w2p", [30, 1], FP16, isOutput=False)
    bo_ext = nc.declare_dram_parameter("boutp", [1, 1], F32, isOutput=False)
    wd_ext = nc.declare_dram_parameter("wday", [128, 90], FP16, isOutput=False)
    out_ext = nc.declare_dram_parameter("out", [1, ROWS], F32, isOutput=True)

    GW = G * NW

    ctx = contextlib.ExitStack()
    with ctx:
        ctx.enter_context(nc.allow_low_precision("fp16 by design"))
        tc = ctx.enter_context(tile.TileContext(nc))
        const = ctx.enter_context(tc.tile_pool(name="const", bufs=1))
        work = ctx.enter_context(tc.tile_pool(name="work", bufs=3))
        hp = ctx.enter_context(tc.tile_pool(name="hp", bufs=2, space="PSUM"))

        wday = const.tile([128, 90], FP16, tag="wday")
        w1b = const.tile([128, 5 * 30], FP16, tag="w1b")
        b1b = const.tile([30, 1], F32, tag="b1b")
        w2b = const.tile([30, 1], FP16, tag="w2b")
        bob = const.tile([1, 1], F32, tag="bob")
        bias_kap = const.tile([128, 1], F32, tag="bias_kap")
        bias_one = const.tile([128, 1], F32, tag="bias_one")
        bias_zero = const.tile([128, 1], F32, tag="bias_zero")
        nc.sync.dma_start(wday[:], wd_ext[:])
        nc.sync.dma_start(
            w1b[:].rearrange("p (k m) -> p k m", k=5, m=30),
            w1_ext[:].rearrange("(k p) m -> p k m", k=5, p=128),
        )
        nc.sync.dma_start(b1b[:], b1_ext[:])
        nc.sync.dma_start(w2b[:], w2_ext[:])
        nc.sync.dma_start(bob[:], bo_ext[:])
        nc.vector.memset(bias_kap[:], KAP)
        nc.vector.memset(bias_one[:], 1.0)
        nc.vector.memset(bias_zero[:], 0.0)

        for it in range(NITER):
            c0 = it * G * CHUNK
            xw = work.tile([128, G * 270], FP16, tag="xw")
            nc.sync.dma_start(
                xw[:].rearrange("p (g q) -> p g q", g=G, q=270),
                x_ext[c0:c0 + G * CHUNK, :].rearrange("(g p) q -> p g q", g=G, p=128),
            )
            # x is w-major: (g, w, f, d)
            xq3 = xw[:].rearrange("p (gw f d) -> p gw f d", gw=GW, f=NF, d=ND)
            xq4 = xw[:].rearrange("p (g w f d) -> p g w f d", g=G, w=NW, f=NF, d=ND)
            xfl = xw[:].rearrange("p (gw q) -> p gw q", gw=GW, q=90)

            P = work.tile([128, GW * NSLOT * ND], FP16, tag="P")
            Pq = P[:].rearrange("p (gw k d) -> p gw k d", gw=GW, k=NSLOT, d=ND)
            Pf = P[:].rearrange("p (gw q) -> p gw q", gw=GW, q=NSLOT * ND)
            Pslot = P[:].rearrange("p (gwk d) -> p gwk d", gwk=GW * NSLOT, d=ND)

            # diag squares -> slots 0:9 (Act)
            act_raw(nc, Pf[:, :, 0:90], xfl, AF.Square, bias_zero[:], 1.0)
            # pair products -> slots 9:45 (DVE, triu order: (i, i+1..8))
            base = 9
            for i in range(NF - 1):
                nj = NF - 1 - i
                nc.vector.tensor_tensor(
                    out=Pq[:, :, base:base + nj, :],
                    in0=xq3[:, :, i + 1:, :],
                    in1=xq3[:, :, i:i + 1, :].broadcast_to([128, GW, nj, ND]),
                    op=ALU.mult,
                )
                base += nj
            # dw products -> slots 45:54 (Pool)
            nc.gpsimd.tensor_tensor(
                out=Pf[:, :, 450:540],
                in0=xfl,
                in1=wday[:].rearrange("p (a q) -> p a q", a=1, q=90)
                    .broadcast_to([128, GW, 90]),
                op=ALU.mult,
            )

            # --- halving-tree d-reduce on all 54 slots (DVE)
            T5 = work.tile([128, GW * NSLOT * 5], FP16, tag="T5")
            T5q = T5[:].rearrange("p (gwk e) -> p gwk e", gwk=GW * NSLOT, e=5)
            nc.vector.tensor_tensor(out=T5q, in0=Pslot[:, :, 0:5],
                                    in1=Pslot[:, :, 5:10], op=ALU.add)
            T2 = work.tile([128, GW * NSLOT * 2], FP16, tag="T2")
            T2q = T2[:].rearrange("p (gwk e) -> p gwk e", gwk=GW * NSLOT, e=2)
            nc.vector.tensor_tensor(out=T2q, in0=T5q[:, :, 0:2], in1=T5q[:, :, 2:4],
                                    op=ALU.add)
            S = work.tile([128, GW * NSLOT], FP16, tag="S")
            nc.vector.tensor_tensor(out=S[:], in0=T2q[:, :, 0], in1=T2q[:, :, 1],
                                    op=ALU.add)
            nc.gpsimd.tensor_tensor(out=S[:], in0=S[:], in1=T5q[:, :, 4], op=ALU.add)
            Sq = S[:].rearrange("p (gw k) -> p gw k", gw=GW, k=NSLOT)
            Sq4 = S[:].rearrange("p (g w k) -> p g w k", g=G, w=NW, k=NSLOT)

            # --- msum tree (Pool), from xw directly
            m5 = work.tile([128, GW * NF * 5], FP16, tag="m5")
            m5q = m5[:].rearrange("p (q e) -> p q e", q=GW * NF, e=5)
            xs = xw[:].rearrange("p (q d) -> p q d", q=GW * NF, d=ND)
            nc.gpsimd.tensor_tensor(out=m5q, in0=xs[:, :, 0:5], in1=xs[:, :, 5:10],
                                    op=ALU.add)
            m2 = work.tile([128, GW * NF * 2], FP16, tag="m2")
            m2q = m2[:].rearrange("p (q e) -> p q e", q=GW * NF, e=2)
            nc.gpsimd.tensor_tensor(out=m2q, in0=m5q[:, :, 0:2], in1=m5q[:, :, 2:4],
                                    op=ALU.add)
            msum = work.tile([128, GW * NF], FP16, tag="msum")
            nc.gpsimd.tensor_tensor(out=msum[:], in0=m2q[:, :, 0], in1=m2q[:, :, 1],
                                    op=ALU.add)
            nc.gpsimd.tensor_tensor(out=msum[:], in0=msum[:], in1=m5q[:, :, 4],
                                    op=ALU.add)
            ms3 = msum[:].rearrange("p (gw f) -> p gw f", gw=GW, f=NF)
            ms4 = msum[:].rearrange("p (g w f) -> p g w f", g=G, w=NW, f=NF)

            F = work.tile([128, G * FPAD], FP16, tag="F")
            Fg = F[:].rearrange("p (g q) -> p g q", g=G, q=FPAD)
            big = Fg[:, :, BIG0:BIG0 + 372].rearrange("p g (w c) -> p g w c", w=NW, c=WBLK)
            sml = big
            if it < 3:
                # zero F pads once per rotating buffer: family pads (col 9 of
                # each 10-block after 72) and the 620:640 tail
                nc.gpsimd.memset(
                    big[:, :, :, 81:121].rearrange("p g w (f o) -> p g w f o", f=4, o=10)
                        [:, :, :, :, 0], 0.0)
                nc.gpsimd.memset(big[:, :, :, 121:124], 0.0)
                nc.gpsimd.memset(Fg[:, :, 620:640], 0.0)

            # m feature: t = 0.1*msum + 1 (Act fused scale+bias) -> F m-cols
            m_cols = sml[:, :, :, SFAM["m"]:SFAM["m"] + 9]
            act_raw(nc, m_cols, ms3, AF.Identity, bias_one[:], 0.1)
            # ZZ s-major: [M16 108 | pad4 | rz 108 | pad4]; M16 = msum*sqrt(10)/10
            ZZ = work.tile([128, 224], FP16, tag="ZZ")
            act_raw(nc, ZZ[:, 0:108], msum[:], AF.Identity, bias_zero[:], SQ10)

            # V = Sd - 0.1*msum^2 (fp16); rz = rsqrt(V + kap)
            msq = work.tile([128, GW * NF], FP16, tag="msq")
            nc.vector.scalar_tensor_tensor(out=msq[:], in0=msum[:], scalar=0.1,
                                           in1=msum[:], op0=ALU.mult, op1=ALU.mult)
            V = work.tile([128, GW * NF], FP16, tag="V")
            V3 = V[:].rearrange("p (gw f) -> p gw f", gw=GW, f=NF)
            nc.vector.tensor_tensor(out=V3, in0=Sq[:, :, 0:9],
                                    in1=msq[:].rearrange("p (gw f) -> p gw f",
                                                         gw=GW, f=NF),
                                    op=ALU.subtract)
            act_raw(nc, ZZ[:, 112:220], V3, AF.Rsqrt, bias_kap[:], 1.0)

            # pair products of [M16 | rz]: mm (s=0), rzp (s=1) (DVE, triu order)
            ZP = work.tile([128, 2 * GW * NPAIR], FP16, tag="ZP")
            ZPq = ZP[:].rearrange("p (s gw k) -> p s gw k", s=2, gw=GW, k=NPAIR)
            ZZp = ZZ[:].rearrange("p (s q) -> p s q", s=2, q=112)
            ZZv = ZZp[:, :, 0:108].rearrange("p s (gw f) -> p s gw f", gw=GW, f=9)
            base = 0
            for i in range(NF - 1):
                nj = NF - 1 - i
                nc.vector.tensor_tensor(
                    out=ZPq[:, :, :, base:base + nj],
                    in0=ZZv[:, :, :, i + 1:9],
                    in1=ZZv[:, :, :, i:i + 1].broadcast_to([128, 2, GW, nj]),
                    op=ALU.mult)
                base += nj

            # cov = S_pairs - mm ; corr = cov * rzp  -> F big blocks
            ZP4 = ZP[:].rearrange("p (s g w k) -> p s g w k", s=2, g=G, w=NW, k=NPAIR)
            nc.vector.tensor_tensor(out=big[:, :, :, 36:72],
                                    in0=Sq4[:, :, :, 9:45],
                                    in1=ZP4[:, 0], op=ALU.subtract)
            nc.vector.tensor_tensor(out=big[:, :, :, 0:36],
                                    in0=big[:, :, :, 36:72],
                                    in1=ZP4[:, 1], op=ALU.mult)

            # sd = V * rz (Pool) ; zs = t * rz (DVE)
            rz4 = ZZ[:, 112:220].rearrange("p (g w f) -> p g w f", g=G, w=NW, f=NF)
            V4 = V[:].rearrange("p (g w f) -> p g w f", g=G, w=NW, f=NF)
            nc.vector.tensor_tensor(out=sml[:, :, :, SFAM["sd"]:SFAM["sd"] + 9],
                                    in0=V4, in1=rz4, op=ALU.mult)
            nc.vector.tensor_tensor(out=sml[:, :, :, SFAM["zs"]:SFAM["zs"] + 9],
                                    in0=m_cols, in1=rz4, op=ALU.mult)

            # ret = (xl + 1) * rsq ; rsq = rr^2 ; rr = rsqrt(xf + 1)  (Act+DVE)
            rr = work.tile([128, GW * NF], FP16, tag="rr")
            rrq = rr[:].rearrange("p (gw f) -> p gw f", gw=GW, f=NF)
            act_raw(nc, rrq, xq3[:, :, :, 0], AF.Rsqrt, bias_one[:], 1.0)
            rsq = work.tile([128, GW * NF], FP16, tag="rsq")
            act_raw(nc, rsq[:], rr[:], AF.Square, bias_zero[:], 1.0)
            rsq4 = rsq[:].rearrange("p (g w f) -> p g w f", g=G, w=NW, f=NF)
            reta = work.tile([128, GW * NF], FP16, tag="reta")
            nc.vector.tensor_tensor(
                out=reta[:].rearrange("p (gw f) -> p gw f", gw=GW, f=NF),
                in0=xq3[:, :, :, 9],
                in1=rsq[:].rearrange("p (gw f) -> p gw f", gw=GW, f=NF),
                op=ALU.mult)
            nc.vector.tensor_tensor(
                out=sml[:, :, :, SFAM["ret"]:SFAM["ret"] + 9],
                in0=reta[:].rearrange("p (g w f) -> p g w f", g=G, w=NW, f=NF),
                in1=rsq4, op=ALU.add)

            # decay: copy dw tree-sums (Act copy)
            nc.scalar.copy(
                out=sml[:, :, :, SFAM["decay"]:SFAM["decay"] + 9],
                in_=Sq4[:, :, :, 45:54])

            # window max/min over 124-wide blocks (DVE)
            t124a = work.tile([128, G * WBLK], FP16, tag="t124a")
            t124b = work.tile([128, G * WBLK], FP16, tag="t124b")
            ta = t124a[:].rearrange("p (g c) -> p g c", g=G, c=WBLK)
            tb = t124b[:].rearrange("p (g c) -> p g c", g=G, c=WBLK)
            nc.vector.tensor_tensor(out=ta, in0=big[:, :, 0, :], in1=big[:, :, 1, :],
                                    op=ALU.max)
            nc.vector.tensor_tensor(out=Fg[:, :, MAX0:MAX0 + WBLK], in0=ta,
                                    in1=big[:, :, 2, :], op=ALU.max)
            nc.vector.tensor_tensor(out=tb, in0=big[:, :, 0, :], in1=big[:, :, 1, :],
                                    op=ALU.min)
            nc.vector.tensor_tensor(out=Fg[:, :, MIN0:MIN0 + WBLK], in0=tb,
                                    in1=big[:, :, 2, :], op=ALU.min)

            # --- transpose F on the DMA engines (XBAR), one call per iter
            Ft = work.tile([128, G * 5 * 128], FP16, tag="Ft")
            Ftv = Ft[:].rearrange("p (g b r) -> p g b r", g=G, b=5, r=128)
            nc.sync.dma_start(
                Ft[:].rearrange("p (gb r) -> p gb r", gb=G * 5, r=128),
                F[:], transpose=True)

            # --- MLP
            h_ps = hp.tile([30, G * 128], F32, tag="h_ps")
            for b in range(5):
                nc.tensor.matmul(out=h_ps[:], lhsT=w1b[:, b * 30:(b + 1) * 30],
                                 rhs=Ftv[:, :, b, :],
                                 start=(b == 0), stop=(b == 4))
            h16 = work.tile([30, G * 128], FP16, tag="h16")
            nc.scalar.activation(out=h16[:], in_=h_ps[:], func=AF.Relu,
                                 bias=b1b[:], scale=1.0)
            o_ps = hp.tile([1, G * 128], F32, tag="o_ps")
            nc.tensor.matmul(out=o_ps[:], lhsT=w2b[:], rhs=h16[:],
                             start=True, stop=True)
            ot = work.tile([1, G * CHUNK], F32, tag="ot")
            nc.scalar.activation(out=ot[:], in_=o_ps[:],
                                 func=AF.Identity, bias=bob[:], scale=1.0)
            nc.sync.dma_start(out_ext[:, c0:c0 + G * CHUNK], ot[:])

    split_multi_waits(nc)
    return nc


# ---------------------------------------------------------------------------
# Host: stored features for the stats sample (mirrors device math), fold.
def host_features(xt16):
    xt = xt16.astype(np.float32).reshape(-1, NW, NF, ND)   # (row, w, f, d) w-major
    xt = np.ascontiguousarray(xt.transpose(0, 2, 1, 3))    # (row, f, w, d)
    f16 = lambda a: a.astype(np.float16).astype(np.float32)
    msum = f16(xt.sum(-1))
    t = f16(0.1 * msum + 1.0)                              # stored m = mean
    M16 = f16(msum * SQ10)
    S = f16((f16(xt[:, I_IDX] * xt[:, J_IDX])).sum(-1, dtype=np.float32))
    Sd = f16(f16(xt * xt).sum(-1, dtype=np.float32))
    mm = f16(M16[:, I_IDX] * M16[:, J_IDX])
    cov = f16(S - mm)
    msq = f16(msum * 0.1 * msum)
    V = f16(Sd - msq)
    rz = f16(1.0 / np.sqrt(V + KAP))
    rzp = f16(rz[:, I_IDX] * rz[:, J_IDX])
    corr = f16(cov * rzp)
    sd = f16(V * rz)
    zs = f16(t * rz)
    xf, xl = xt[..., 0], xt[..., -1]
    rr = f16(1.0 / np.sqrt(xf + 1.0))
    rsq = f16(rr * rr)
    ret = f16((xl + 1.0) * rsq)
    wd = np.arange(1, ND + 1, dtype=np.float32)
    decay = f16(f16(xt * wd).sum(-1, dtype=np.float32))
    F = np.concatenate([corr, cov, sd, zs, ret, decay, t], axis=1)  # [n,117,3]
    mx = f16(np.maximum(np.maximum(F[..., 0], F[..., 1]), F[..., 2]))
    mn = f16(np.minimum(np.minimum(F[..., 0], F[..., 1]), F[..., 2]))
    usum = f16(f16(F[..., 0] + F[..., 1]) + F[..., 2])
    return F, mx, mn, usum


def fold(x16_sample, gamma, beta, W1, b1, W2, b2, w_out, b_out):
    F, mx, mn, usum = host_features(x16_sample)
    n = F.shape[0]
    cols = np.concatenate([F.reshape(n, -1), mx, mn, usum], axis=1).astype(np.float64)
    s1 = cols.sum(0)
    s2 = (cols ** 2).sum(0)

    alpha = np.zeros(351)
    bet = np.zeros(351)
    a_t = np.zeros(117)
    b_t = np.zeros(117)
    f0 = 0
    for gname, sz in GROUPS:
        s, c = S_C[gname]
        cs = slice(f0 * 3, (f0 + sz) * 3)
        e1 = s1[cs].sum() / (n * sz * 3)
        e2 = s2[cs].sum() / (n * sz * 3)
        mean_ref = s * e1 + c
        var_ref = s * s * (e2 - e1 * e1)
        a = gamma / np.sqrt(var_ref + EPS_BN)
        alpha[cs] = a * s
        bet[cs] = beta + a * (c - mean_ref)
        a_t[f0:f0 + sz] = a * s
        b_t[f0:f0 + sz] = beta + a * (c - mean_ref)
        f0 += sz

    def stage2(bs1, bs2, scale):
        e1 = bs1 / n
        e2 = bs2 / n
        mean_all = (a_t * scale * e1 + b_t).mean()
        ex2_all = ((a_t * scale) ** 2 * e2 + 2 * a_t * scale * b_t * e1 + b_t ** 2).mean()
        a2 = gamma / np.sqrt(ex2_all - mean_all ** 2 + EPS_BN)
        return a2, beta - a2 * mean_all

    a2x, b2x = stage2(s1[351:468], s2[351:468], 1.0)
    a2n, b2n = stage2(s1[468:585], s2[468:585], 1.0)
    a2m, b2m = stage2(s1[585:702], s2[585:702], 1.0 / 3.0)

    W1 = W1.astype(np.float64)
    Wx, Wm, WX, WN = W1[:, 0:351], W1[:, 351:468], W1[:, 468:585], W1[:, 585:702]
    W_xcat = (Wx * alpha[None, :]).reshape(30, 117, 3) \
        + (Wm * (a2m * a_t / 3.0)[None, :])[:, :, None]     # [30, f, w]
    W_max = WX * (a2x * a_t)[None, :]
    W_min = WN * (a2n * a_t)[None, :]
    b_eff = (b1.astype(np.float64) + Wx @ bet + Wm @ (a2m * b_t + b2m)
             + WX @ (a2x * b_t + b2x) + WN @ (a2n * b_t + b2n))

    # permute into device F layout [640]
    def blk_col(fg):
        if fg < 72:
            return fg                          # corr k -> k, cov k -> 36+k
        fam, k = divmod(fg - 72, 9)
        return 72 + 10 * fam + k

    Wd = np.zeros((30, FPAD))
    for fg in range(117):
        for w in range(3):
            Wd[:, BIG0 + w * WBLK + blk_col(fg)] = W_xcat[:, fg, w]
        Wd[:, MAX0 + blk_col(fg)] = W_max[:, fg]
        Wd[:, MIN0 + blk_col(fg)] = W_min[:, fg]

    w1t = np.ascontiguousarray(Wd.T).astype(np.float16)     # [640, 30]
    b1p = b_eff.reshape(30, 1).astype(np.float32)
    w2p = (W2.reshape(-1) * float(np.asarray(w_out).reshape(-1)[0])).reshape(30, 1).astype(np.float16)
    boutp = np.array([[float(np.asarray(b2).reshape(-1)[0]) * float(np.asarray(w_out).reshape(-1)[0])
                       + float(np.asarray(b_out).reshape(-1)[0])]], np.float32)
    return w1t, b1p, w2p, boutp


_CACHE = {}


def kernel(xb, gamma, beta, W1, b1, W2, b2, w_out, b_out):
    x16 = (np.asarray(xb, np.float32).reshape(B_TOTAL, 270) - 1.0).astype(np.float16)
    # device layout: w-major (row, w, f, d)
    x16 = np.ascontiguousarray(
        x16.reshape(B_TOTAL, NF, NW, ND).transpose(0, 2, 1, 3)).reshape(B_TOTAL, 270)
    # stats sample: first 2048 rows of each shard (w-major, as host_features expects)
    samp = np.concatenate([x16[i * ROWS:i * ROWS + 2048] for i in range(NCORES)])
    w1t, b1p, w2p, boutp = fold(
        samp, float(np.asarray(gamma).reshape(-1)[0]), float(np.asarray(beta).reshape(-1)[0]),
        np.asarray(W1, np.float64), np.asarray(b1, np.float64),
        np.asarray(W2, np.float64), b2, w_out, b_out)

    if "nc" not in _CACHE:
        _CACHE["nc"] = build_neff()
    nc = _CACHE["nc"]
    wd = np.tile(np.arange(1, ND + 1, dtype=np.float16)[None, :], (128, NF))
    in_maps = [
        {"x16": np.ascontiguousarray(x16[i * ROWS:(i + 1) * ROWS]),
         "w1t": w1t, "b1p": b1p, "w2p": w2p, "boutp": boutp, "wday": wd}
        for i in range(NCORES)
    ]
    res = run_bass_kernel_spmd(nc, in_maps, core_ids=list(range(NCORES)))
    out = np.concatenate([res.results[i]["out"].reshape(-1) for i in range(NCORES)])
    return out.astype(np.float32)


# revision 14
# speedup vs baseline: 1.0621x; 1.0621x over previous
"""AlphaNet_v1 Trainium2 kernel — single fused NEFF, 8-core data parallel.

Host side: x~ = (x-1) as fp16 (centering shifts are BN-invariant), w-major
(row, w, f, d); BN stats from a 16384-row sample via numpy; BN folded into
MLP weights; W1 permuted to the device F-column layout.

Device (per core, per 512-row iteration, all fp16):
  P [128,(g,w,54,10)] products: diag squares (Act), 36 pair products (DVE,
  8 offset-batched TT), day-weight products (Pool). Halving-tree d-reduce
  (DVE, final stage Pool) -> S [g,w,54]. msum tree on DVE. Derived
  features with fused tensor_scalar / scalar_tensor_tensor ops into a
  4B-aligned (w,124)-blocked F [128, 640] per chunk; window max/min on
  DVE. F transposed by the DMA engines (XBAR transpose, one call/iter)
  into Ft [128,(g,5,128)]; fused MLP on PE (5 accumulating matmuls +
  relu + W2) -> out [16384] f32.

Stored-feature units (ref = s*stored + c): corr (1,0), cov (1/9,0),
sd=V*rsqrt(V+kap) (1/3,0), zs=mean*rsqrt(V+kap) (3,0), ret=xl/xf (1,-1),
decay (1/55,+1), m=mean (1,0).
"""

import contextlib
import numpy as np

import bass_rust
import concourse.bass as bass
import concourse.mybir as mybir
import concourse.tile as tile
from concourse.bass_utils import run_bass_kernel_spmd

F32 = mybir.dt.float32
FP16 = mybir.dt.float16
ALU = mybir.AluOpType
AF = mybir.ActivationFunctionType
AX = mybir.AxisListType

NCORES = 8
B_TOTAL = 131072
ROWS = B_TOTAL // NCORES          # 16384
CHUNK = 128
G = 4                             # chunks per iteration
NITER = ROWS // (CHUNK * G)       # 32
NF, NW, ND = 9, 3, 10
NPAIR = 36
NSLOT = 54                        # per (row, window): diag 0:9 | pairs 9:45 | dw 45:54
FPAD = 640
EPS_BN, EPS = 1e-5, 1e-8
KAP = 1e-6
SQ10 = float(np.sqrt(10.0) / 10.0)

I_IDX, J_IDX = np.triu_indices(NF, k=1)
GROUPS = [("corr", 36), ("cov", 36), ("sd", 9), ("zs", 9), ("ret", 9), ("decay", 9), ("m", 9)]
S_C = {"corr": (1.0, 0.0), "cov": (1.0 / 9.0, 0.0), "sd": (1.0 / 3.0, 0.0),
       "zs": (3.0, 0.0), "ret": (1.0, -1.0), "decay": (1.0 / 55.0, 1.0), "m": (1.0, 0.0)}
# F column layout per chunk (all starts 4B-aligned):
#  0:372    3x (w,124) [corr36 | cov36 | sd10 | zs10 | ret10 | dec10 | m10 | pad2]
#  372:496 max124 | 496:620 min124 | 620:640 pad
WBLK = 124
BIG0, MAX0, MIN0 = 0, 372, 496
SFAM = {"sd": 72, "zs": 82, "ret": 92, "decay": 102, "m": 112}

_wsplit_n = [0]


def split_multi_waits(nc):
    for fn in nc.m.functions:
        for bb in fn.blocks:
            new_list = []
            for ins in bb.instructions:
                si = ins.sync_info
                waits = list(si.on_wait) if (si is not None and si.on_wait) else []
                if len(waits) > 1:
                    for w in waits[:-1]:
                        _wsplit_n[0] += 1
                        d = bass_rust.InstDrain(
                            name=f"wsplit-{_wsplit_n[0]}", ins=[], outs=[]
                        )
                        d.engine = ins.engine
                        d.sync_info = mybir.SyncInfo(on_wait=[w], on_update=[])
                        new_list.append(d)
                    si.on_wait = [waits[-1]]
                new_list.append(ins)
            bb.instructions[:] = new_list


def act_raw(nc, out, in_, func, bias_ap, scale):
    sc = nc.scalar
    ins = [
        sc.lower_ap(in_),
        sc.lower_ap(bias_ap),
        mybir.ImmediateValue(dtype=mybir.dt.float32, value=float(scale)),
        mybir.ImmediateValue(dtype=mybir.dt.float32, value=0.0),
    ]
    return sc.add_instruction(
        mybir.InstActivation(
            name=nc.get_next_instruction_name(),
            func=func,
            ins=ins,
            outs=[sc.lower_ap(out)],
        )
    )


# ---------------------------------------------------------------------------
def build_neff():
    nc = bass.Bass()
    x_ext = nc.declare_dram_parameter("x16", [ROWS, 270], FP16, isOutput=False)
    w1_ext = nc.declare_dram_parameter("w1t", [FPAD, 30], FP16, isOutput=False)
    b1_ext = nc.declare_dram_parameter("b1p", [30, 1], F32, isOutput=False)
    w2_ext = nc.declare_dram_parameter("w2p", [30, 1], FP16, isOutput=False)
    bo_ext = nc.declare_dram_parameter("boutp", [1, 1], F32, isOutput=False)
    wd_ext = nc.declare_dram_parameter("wday", [128, 90], FP16, isOutput=False)
    out_ext = nc.declare_dram_parameter("out", [1, ROWS], F32, isOutput=True)

    GW = G * NW

    ctx = contextlib.ExitStack()
    with ctx:
        ctx.enter_context(nc.allow_low_precision("fp16 by design"))
        tc = ctx.enter_context(tile.TileContext(nc))
        const = ctx.enter_context(tc.tile_pool(name="const", bufs=1))
        work = ctx.enter_context(tc.tile_pool(name="work", bufs=3))
        hp = ctx.enter_context(tc.tile_pool(name="hp", bufs=2, space="PSUM"))

        wday = const.tile([128, 90], FP16, tag="wday")
        w1b = const.tile([128, 5 * 30], FP16, tag="w1b")
        b1b = const.tile([30, 1], F32, tag="b1b")
        w2b = const.tile([30, 1], FP16, tag="w2b")
        bob = const.tile([1, 1], F32, tag="bob")
        bias_kap = const.tile([128, 1], F32, tag="bias_kap")
        bias_one = const.tile([128, 1], F32, tag="bias_one")
        bias_zero = const.tile([128, 1], F32, tag="bias_zero")
        nc.sync.dma_start(wday[:], wd_ext[:])
        nc.sync.dma_start(
            w1b[:].rearrange("p (k m) -> p k m", k=5, m=30),
            w1_ext[:].rearrange("(k p) m -> p k m", k=5, p=128),
        )
        nc.sync.dma_start(b1b[:], b1_ext[:])
        nc.sync.dma_start(w2b[:], w2_ext[:])
        nc.sync.dma_start(bob[:], bo_ext[:])
        nc.vector.memset(bias_kap[:], KAP)
        nc.vector.memset(bias_one[:], 1.0)
        nc.vector.memset(bias_zero[:], 0.0)

        for it in range(NITER):
            c0 = it * G * CHUNK
            xw = work.tile([128, G * 270], FP16, tag="xw")
            nc.sync.dma_start(
                xw[:].rearrange("p (g q) -> p g q", g=G, q=270),
                x_ext[c0:c0 + G * CHUNK, :].rearrange("(g p) q -> p g q", g=G, p=128),
            )
            # x is w-major: (g, w, f, d)
            xq3 = xw[:].rearrange("p (gw f d) -> p gw f d", gw=GW, f=NF, d=ND)
            xq4 = xw[:].rearrange("p (g w f d) -> p g w f d", g=G, w=NW, f=NF, d=ND)
            xfl = xw[:].rearrange("p (gw q) -> p gw q", gw=GW, q=90)

            P = work.tile([128, GW * NSLOT * ND], FP16, tag="P")
            Pq = P[:].rearrange("p (gw k d) -> p gw k d", gw=GW, k=NSLOT, d=ND)
            Pf = P[:].rearrange("p (gw q) -> p gw q", gw=GW, q=NSLOT * ND)
            Pslot = P[:].rearrange("p (gwk d) -> p gwk d", gwk=GW * NSLOT, d=ND)

            # diag squares -> slots 0:9 (Act)
            act_raw(nc, Pf[:, :, 0:90], xfl, AF.Square, bias_zero[:], 1.0)
            # pair products -> slots 9:45 (DVE, triu order: (i, i+1..8))
            base = 9
            for i in range(NF - 1):
                nj = NF - 1 - i
                nc.vector.tensor_tensor(
                    out=Pq[:, :, base:base + nj, :],
                    in0=xq3[:, :, i + 1:, :],
                    in1=xq3[:, :, i:i + 1, :].broadcast_to([128, GW, nj, ND]),
                    op=ALU.mult,
                )
                base += nj
            # dw products -> slots 45:54 (Pool)
            nc.gpsimd.tensor_tensor(
                out=Pf[:, :, 450:540],
                in0=xfl,
                in1=wday[:].rearrange("p (a q) -> p a q", a=1, q=90)
                    .broadcast_to([128, GW, 90]),
                op=ALU.mult,
            )

            # --- halving-tree d-reduce on all 54 slots (DVE)
            T5 = work.tile([128, GW * NSLOT * 5], FP16, tag="T5")
            T5q = T5[:].rearrange("p (gwk e) -> p gwk e", gwk=GW * NSLOT, e=5)
            nc.vector.tensor_tensor(out=T5q, in0=Pslot[:, :, 0:5],
                                    in1=Pslot[:, :, 5:10], op=ALU.add)
            T2 = work.tile([128, GW * NSLOT * 2], FP16, tag="T2")
            T2q = T2[:].rearrange("p (gwk e) -> p gwk e", gwk=GW * NSLOT, e=2)
            nc.vector.tensor_tensor(out=T2q, in0=T5q[:, :, 0:2], in1=T5q[:, :, 2:4],
                                    op=ALU.add)
            S = work.tile([128, GW * NSLOT], FP16, tag="S")
            nc.vector.tensor_tensor(out=S[:], in0=T2q[:, :, 0], in1=T2q[:, :, 1],
                                    op=ALU.add)
            nc.gpsimd.tensor_tensor(out=S[:], in0=S[:], in1=T5q[:, :, 4], op=ALU.add)
            Sq = S[:].rearrange("p (gw k) -> p gw k", gw=GW, k=NSLOT)
            Sq4 = S[:].rearrange("p (g w k) -> p g w k", g=G, w=NW, k=NSLOT)

            # --- msum tree (Pool), from xw directly
            m5 = work.tile([128, GW * NF * 5], FP16, tag="m5")
            m5q = m5[:].rearrange("p (q e) -> p q e", q=GW * NF, e=5)
            xs = xw[:].rearrange("p (q d) -> p q d", q=GW * NF, d=ND)
            nc.vector.tensor_tensor(out=m5q, in0=xs[:, :, 0:5], in1=xs[:, :, 5:10],
                                    op=ALU.add)
            m2 = work.tile([128, GW * NF * 2], FP16, tag="m2")
            m2q = m2[:].rearrange("p (q e) -> p q e", q=GW * NF, e=2)
            nc.vector.tensor_tensor(out=m2q, in0=m5q[:, :, 0:2], in1=m5q[:, :, 2:4],
                                    op=ALU.add)
            msum = work.tile([128, GW * NF], FP16, tag="msum")
            nc.vector.tensor_tensor(out=msum[:], in0=m2q[:, :, 0], in1=m2q[:, :, 1],
                                    op=ALU.add)
            nc.vector.tensor_tensor(out=msum[:], in0=msum[:], in1=m5q[:, :, 4],
                                    op=ALU.add)
            ms3 = msum[:].rearrange("p (gw f) -> p gw f", gw=GW, f=NF)
            ms4 = msum[:].rearrange("p (g w f) -> p g w f", g=G, w=NW, f=NF)

            F = work.tile([128, G * FPAD], FP16, tag="F")
            Fg = F[:].rearrange("p (g q) -> p g q", g=G, q=FPAD)
            big = Fg[:, :, BIG0:BIG0 + 372].rearrange("p g (w c) -> p g w c", w=NW, c=WBLK)
            sml = big
            if it < 3:
                # zero F pads once per rotating buffer: family pads (col 9 of
                # each 10-block after 72) and the 620:640 tail
                nc.gpsimd.memset(
                    big[:, :, :, 81:121].rearrange("p g w (f o) -> p g w f o", f=4, o=10)
                        [:, :, :, :, 0], 0.0)
                nc.gpsimd.memset(big[:, :, :, 121:124], 0.0)
                nc.gpsimd.memset(Fg[:, :, 620:640], 0.0)

            # m feature: t = 0.1*msum + 1 (Act fused scale+bias) -> F m-cols
            m_cols = sml[:, :, :, SFAM["m"]:SFAM["m"] + 9]
            act_raw(nc, m_cols, ms3, AF.Identity, bias_one[:], 0.1)
            # ZZ s-major: [M16 108 | pad4 | rz 108 | pad4]; M16 = msum*sqrt(10)/10
            ZZ = work.tile([128, 224], FP16, tag="ZZ")
            act_raw(nc, ZZ[:, 0:108], msum[:], AF.Identity, bias_zero[:], SQ10)

            # V = Sd - 0.1*msum^2 (fp16); rz = rsqrt(V + kap)
            msq = work.tile([128, GW * NF], FP16, tag="msq")
            nc.vector.scalar_tensor_tensor(out=msq[:], in0=msum[:], scalar=0.1,
                                           in1=msum[:], op0=ALU.mult, op1=ALU.mult)
            V = work.tile([128, GW * NF], FP16, tag="V")
            V3 = V[:].rearrange("p (gw f) -> p gw f", gw=GW, f=NF)
            nc.vector.tensor_tensor(out=V3, in0=Sq[:, :, 0:9],
                                    in1=msq[:].rearrange("p (gw f) -> p gw f",
                                                         gw=GW, f=NF),
                                    op=ALU.subtract)
            act_raw(nc, ZZ[:, 112:220], V3, AF.Rsqrt, bias_kap[:], 1.0)

            # pair products of [M16 | rz]: mm (s=0), rzp (s=1) (DVE, triu order)
            ZP = work.tile([128, 2 * GW * NPAIR], FP16, tag="ZP")
            ZPq = ZP[:].rearrange("p (s gw k) -> p s gw k", s=2, gw=GW, k=NPAIR)
            ZZp = ZZ[:].rearrange("p (s q) -> p s q", s=2, q=112)
            ZZv = ZZp[:, :, 0:108].rearrange("p s (gw f) -> p s gw f", gw=GW, f=9)
            base = 0
            for i in range(NF - 1):
                nj = NF - 1 - i
                nc.vector.tensor_tensor(
                    out=ZPq[:, :, :, base:base + nj],
                    in0=ZZv[:, :, :, i + 1:9],
                    in1=ZZv[:, :, :, i:i + 1].broadcast_to([128, 2, GW, nj]),
                    op=ALU.mult)
                base += nj

            # cov = S_pairs - mm ; corr = cov * rzp  -> F big blocks
            ZP4 = ZP[:].rearrange("p (s g w k) -> p s g w k", s=2, g=G, w=NW, k=NPAIR)
            nc.vector.tensor_tensor(out=big[:, :, :, 36:72],
                                    in0=Sq4[:, :, :, 9:45],
                                    in1=ZP4[:, 0], op=ALU.subtract)
            nc.vector.tensor_tensor(out=big[:, :, :, 0:36],
                                    in0=big[:, :, :, 36:72],
                                    in1=ZP4[:, 1], op=ALU.mult)

            # sd = V * rz (Pool) ; zs = t * rz (DVE)
            rz4 = ZZ[:, 112:220].rearrange("p (g w f) -> p g w f", g=G, w=NW, f=NF)
            V4 = V[:].rearrange("p (g w f) -> p g w f", g=G, w=NW, f=NF)
            nc.vector.tensor_tensor(out=sml[:, :, :, SFAM["sd"]:SFAM["sd"] + 9],
                                    in0=V4, in1=rz4, op=ALU.mult)
            nc.vector.tensor_tensor(out=sml[:, :, :, SFAM["zs"]:SFAM["zs"] + 9],
                                    in0=m_cols, in1=rz4, op=ALU.mult)

            # ret = (xl + 1) * rsq ; rsq = rr^2 ; rr = rsqrt(xf + 1)  (Act+DVE)
            rr = work.tile([128, GW * NF], FP16, tag="rr")
            rrq = rr[:].rearrange("p (gw f) -> p gw f", gw=GW, f=NF)
            act_raw(nc, rrq, xq3[:, :, :, 0], AF.Rsqrt, bias_one[:], 1.0)
            rsq = work.tile([128, GW * NF], FP16, tag="rsq")
            act_raw(nc, rsq[:], rr[:], AF.Square, bias_zero[:], 1.0)
            rsq4 = rsq[:].rearrange("p (g w f) -> p g w f", g=G, w=NW, f=NF)
            reta = work.tile([128, GW * NF], FP16, tag="reta")
            nc.vector.tensor_tensor(
                out=reta[:].rearrange("p (gw f) -> p gw f", gw=GW, f=NF),
                in0=xq3[:, :, :, 9],
                in1=rsq[:].rearrange("p (gw f) -> p gw f", gw=GW, f=NF),
                op=ALU.mult)
            nc.vector.tensor_tensor(
                out=sml[:, :, :, SFAM["ret"]:SFAM["ret"] + 9],
                in0=reta[:].rearrange("p (g w f) -> p g w f", g=G, w=NW, f=NF),
                in1=rsq4, op=ALU.add)

            # decay: copy dw tree-sums (Act copy)
            nc.scalar.copy(
                out=sml[:, :, :, SFAM["decay"]:SFAM["decay"] + 9],
                in_=Sq4[:, :, :, 45:54])

            # window max/min over 124-wide blocks (DVE)
            t124a = work.tile([128, G * WBLK], FP16, tag="t124a")
            t124b = work.tile([128, G * WBLK], FP16, tag="t124b")
            ta = t124a[:].rearrange("p (g c) -> p g c", g=G, c=WBLK)
            tb = t124b[:].rearrange("p (g c) -> p g c", g=G, c=WBLK)
            nc.vector.tensor_tensor(out=ta, in0=big[:, :, 0, :], in1=big[:, :, 1, :],
                                    op=ALU.max)
            nc.vector.tensor_tensor(out=Fg[:, :, MAX0:MAX0 + WBLK], in0=ta,
                                    in1=big[:, :, 2, :], op=ALU.max)
            nc.vector.tensor_tensor(out=tb, in0=big[:, :, 0, :], in1=big[:, :, 1, :],
                                    op=ALU.min)
            nc.vector.tensor_tensor(out=Fg[:, :, MIN0:MIN0 + WBLK], in0=tb,
                                    in1=big[:, :, 2, :], op=ALU.min)

            # --- transpose F on the DMA engines (XBAR), one call per iter
            Ft = work.tile([128, G * 5 * 128], FP16, tag="Ft")
            Ftv = Ft[:].rearrange("p (g b r) -> p g b r", g=G, b=5, r=128)
            nc.sync.dma_start(
                Ft[:].rearrange("p (gb r) -> p gb r", gb=G * 5, r=128),
                F[:], transpose=True)

            # --- MLP
            h_ps = hp.tile([30, G * 128], F32, tag="h_ps")
            for b in range(5):
                nc.tensor.matmul(out=h_ps[:], lhsT=w1b[:, b * 30:(b + 1) * 30],
                                 rhs=Ftv[:, :, b, :],
                                 start=(b == 0), stop=(b == 4))
            h16 = work.tile([30, G * 128], FP16, tag="h16")
            nc.scalar.activation(out=h16[:], in_=h_ps[:], func=AF.Relu,
                                 bias=b1b[:], scale=1.0)
            o_ps = hp.tile([1, G * 128], F32, tag="o_ps")
            nc.tensor.matmul(out=o_ps[:], lhsT=w2b[:], rhs=h16[:],
                             start=True, stop=True)
            ot = work.tile([1, G * CHUNK], F32, tag="ot")
            nc.scalar.activation(out=ot[:], in_=o_ps[:],
                                 func=AF.Identity, bias=bob[:], scale=1.0)
            nc.sync.dma_start(out_ext[:, c0:c0 + G * CHUNK], ot[:])

    split_multi_waits(nc)
    return nc


# ---------------------------------------------------------------------------
# Host: stored features for the stats sample (mirrors device math), fold.
def host_features(xt16):
    xt = xt16.astype(np.float32).reshape(-1, NW, NF, ND)   # (row, w, f, d) w-major
    xt = np.ascontiguousarray(xt.transpose(0, 2, 1, 3))    # (row, f, w, d)
    f16 = lambda a: a.astype(np.float16).astype(np.float32)
    msum = f16(xt.sum(-1))
    t = f16(0.1 * msum + 1.0)                              # stored m = mean
    M16 = f16(msum * SQ10)
    S = f16((f16(xt[:, I_IDX] * xt[:, J_IDX])).sum(-1, dtype=np.float32))
    Sd = f16(f16(xt * xt).sum(-1, dtype=np.float32))
    mm = f16(M16[:, I_IDX] * M16[:, J_IDX])
    cov = f16(S - mm)
    msq = f16(msum * 0.1 * msum)
    V = f16(Sd - msq)
    rz = f16(1.0 / np.sqrt(V + KAP))
    rzp = f16(rz[:, I_IDX] * rz[:, J_IDX])
    corr = f16(cov * rzp)
    sd = f16(V * rz)
    zs = f16(t * rz)
    xf, xl = xt[..., 0], xt[..., -1]
    rr = f16(1.0 / np.sqrt(xf + 1.0))
    rsq = f16(rr * rr)
    ret = f16((xl + 1.0) * rsq)
    wd = np.arange(1, ND + 1, dtype=np.float32)
    decay = f16(f16(xt * wd).sum(-1, dtype=np.float32))
    F = np.concatenate([corr, cov, sd, zs, ret, decay, t], axis=1)  # [n,117,3]
    mx = f16(np.maximum(np.maximum(F[..., 0], F[..., 1]), F[..., 2]))
    mn = f16(np.minimum(np.minimum(F[..., 0], F[..., 1]), F[..., 2]))
    usum = f16(f16(F[..., 0] + F[..., 1]) + F[..., 2])
    return F, mx, mn, usum


def fold(x16_sample, gamma, beta, W1, b1, W2, b2, w_out, b_out):
    F, mx, mn, usum = host_features(x16_sample)
    n = F.shape[0]
    cols = np.concatenate([F.reshape(n, -1), mx, mn, usum], axis=1).astype(np.float64)
    s1 = cols.sum(0)
    s2 = (cols ** 2).sum(0)

    alpha = np.zeros(351)
    bet = np.zeros(351)
    a_t = np.zeros(117)
    b_t = np.zeros(117)
    f0 = 0
    for gname, sz in GROUPS:
        s, c = S_C[gname]
        cs = slice(f0 * 3, (f0 + sz) * 3)
        e1 = s1[cs].sum() / (n * sz * 3)
        e2 = s2[cs].sum() / (n * sz * 3)
        mean_ref = s * e1 + c
        var_ref = s * s * (e2 - e1 * e1)
        a = gamma / np.sqrt(var_ref + EPS_BN)
        alpha[cs] = a * s
        bet[cs] = beta + a * (c - mean_ref)
        a_t[f0:f0 + sz] = a * s
        b_t[f0:f0 + sz] = beta + a * (c - mean_ref)
        f0 += sz

    def stage2(bs1, bs2, scale):
        e1 = bs1 / n
        e2 = bs2 / n
        mean_all = (a_t * scale * e1 + b_t).mean()
        ex2_all = ((a_t * scale) ** 2 * e2 + 2 * a_t * scale * b_t * e1 + b_t ** 2).mean()
        a2 = gamma / np.sqrt(ex2_all - mean_all ** 2 + EPS_BN)
        return a2, beta - a2 * mean_all

    a2x, b2x = stage2(s1[351:468], s2[351:468], 1.0)
    a2n, b2n = stage2(s1[468:585], s2[468:585], 1.0)
    a2m, b2m = stage2(s1[585:702], s2[585:702], 1.0 / 3.0)

    W1 = W1.astype(np.float64)
    Wx, Wm, WX, WN = W1[:, 0:351], W1[:, 351:468], W1[:, 468:585], W1[:, 585:702]
    W_xcat = (Wx * alpha[None, :]).reshape(30, 117, 3) \
        + (Wm * (a2m * a_t / 3.0)[None, :])[:, :, None]     # [30, f, w]
    W_max = WX * (a2x * a_t)[None, :]
    W_min = WN * (a2n * a_t)[None, :]
    b_eff = (b1.astype(np.float64) + Wx @ bet + Wm @ (a2m * b_t + b2m)
             + WX @ (a2x * b_t + b2x) + WN @ (a2n * b_t + b2n))

    # permute into device F layout [640]
    def blk_col(fg):
        if fg < 72:
            return fg                          # corr k -> k, cov k -> 36+k
        fam, k = divmod(fg - 72, 9)
        return 72 + 10 * fam + k

    Wd = np.zeros((30, FPAD))
    for fg in range(117):
        for w in range(3):
            Wd[:, BIG0 + w * WBLK + blk_col(fg)] = W_xcat[:, fg, w]
        Wd[:, MAX0 + blk_col(fg)] = W_max[:, fg]
        Wd[:, MIN0 + blk_col(fg)] = W_min[:, fg]

    w1t = np.ascontiguousarray(Wd.T).astype(np.float16)     # [640, 30]
    b1p = b_eff.reshape(30, 1).astype(np.float32)
    w2p = (W2.reshape(-1) * float(np.asarray(w_out).reshape(-1)[0])).reshape(30, 1).astype(np.float16)
    boutp = np.array([[float(np.asarray(b2).reshape(-1)[0]) * float(np.asarray(w_out).reshape(-1)[0])
                       + float(np.asarray(b_out).reshape(-1)[0])]], np.float32)
    return w1t, b1p, w2p, boutp


_CACHE = {}


def kernel(xb, gamma, beta, W1, b1, W2, b2, w_out, b_out):
    x16 = (np.asarray(xb, np.float32).reshape(B_TOTAL, 270) - 1.0).astype(np.float16)
    # device layout: w-major (row, w, f, d)
    x16 = np.ascontiguousarray(
        x16.reshape(B_TOTAL, NF, NW, ND).transpose(0, 2, 1, 3)).reshape(B_TOTAL, 270)
    # stats sample: first 2048 rows of each shard (w-major, as host_features expects)
    samp = np.concatenate([x16[i * ROWS:i * ROWS + 2048] for i in range(NCORES)])
    w1t, b1p, w2p, boutp = fold(
        samp, float(np.asarray(gamma).reshape(-1)[0]), float(np.asarray(beta).reshape(-1)[0]),
        np.asarray(W1, np.float64), np.asarray(b1, np.float64),
        np.asarray(W2, np.float64), b2, w_out, b_out)

    if "nc" not in _CACHE:
        _CACHE["nc"] = build_neff()
    nc = _CACHE["nc"]
    wd = np.tile(np.arange(1, ND + 1, dtype=np.float16)[None, :], (128, NF))
    in_maps = [
        {"x16": np.ascontiguousarray(x16[i * ROWS:(i + 1) * ROWS]),
         "w1t": w1t, "b1p": b1p, "w2p": w2p, "boutp": boutp, "wday": wd}
        for i in range(NCORES)
    ]
    res = run_bass_kernel_spmd(nc, in_maps, core_ids=list(range(NCORES)))
    out = np.concatenate([res.results[i]["out"].reshape(-1) for i in range(NCORES)])
    return out.astype(np.float32)
